# revision 6
# baseline (speedup 1.0000x reference)
"""Single-head attention (B=4, S=2048, D=1024) on 8 Trainium2 NeuronCores.

Sharding: core c handles batch b = c//2, query half h = c%2 (1024 queries).
K/V for the full sequence are computed redundantly by both cores of a batch
pair (cheaper than cross-core collectives at this size).

Math notes (exact rewrites of the reference):
  - scores row-softmax is invariant to adding a per-row constant, so the
    K-projection bias `bk` contributes Q@bk (constant along the key axis)
    and drops out entirely.
  - attn rows sum to 1, so the V bias `bv` is a constant additive term on
    the output: out = attn @ V_nobias + bv.
  - softmax is computed without max-subtraction: scores/32 has |s| < ~4 for
    this problem (checked host-side), exp() is well-conditioned there.
  - keys/values are processed in a per-core permuted order (the core's own
    query half of the sequence first); softmax+attn@V is permutation
    invariant along the key axis.

Device pipeline per core (all matmul operands bf16, PSUM accumulation fp32):
  Phase A: Qt[e,q] = WqT.T @ xT (+bq via ACT bias), Kt[e,s] = WkT.T @ xT,
           V[s,e]  = xT.T @ WvT   (PE; ACT drains PSUM->SBUF as bf16)
  Phase B: per 128-query tile: scores = Qt.T @ Kt -> exp(s/32) on ACT with
           fused row-sum (accum_out); PE-transpose of attn tiles; out
           accum = attnT.T @ V; ACT applies 1/rowsum, DVE adds bv; DMA out.
"""

import numpy as np
import ml_dtypes

from contextlib import ExitStack

import concourse.bass as bass
import concourse.mybir as mybir
import concourse.tile as tile
from concourse import bacc
from concourse.masks import make_identity

BF16 = mybir.dt.bfloat16
F32 = mybir.dt.float32
NPBF16 = ml_dtypes.bfloat16

B, S, D = 4, 2048, 1024
NCORES = 8
SQ = S // 2            # queries per core
P = 128                # partitions
NDT = D // P           # 8 d-tiles (input feature dim)
NET = D // P           # 8 e-tiles (projected dim)
NST = S // P           # 16 key tiles
NQT = SQ // P          # 8 query tiles per core
NKC = S // 512         # 4 key chunks of 512
NQC = SQ // 512        # 2 query chunks of 512
NEC = D // 512         # 2 embed chunks of 512
SCALE = 1.0 / 32.0     # 1/sqrt(D)

AF = mybir.ActivationFunctionType

_PROGRAM = None


def _build_program():
    nc = bacc.Bacc(
        "TRN2", target_bir_lowering=False, debug=False, num_devices=NCORES
    )
    xt_d = nc.dram_tensor("xt", [D, S], BF16, kind="ExternalInput")
    wq_d = nc.dram_tensor("wq", [D, D], BF16, kind="ExternalInput")
    wk_d = nc.dram_tensor("wk", [D, D], BF16, kind="ExternalInput")
    wv_d = nc.dram_tensor("wv", [D, D], BF16, kind="ExternalInput")
    bq_d = nc.dram_tensor("bq", [P, NET], F32, kind="ExternalInput")
    bv_d = nc.dram_tensor("bv", [1, D], F32, kind="ExternalInput")
    out_d = nc.dram_tensor("out", [SQ, D], F32, kind="ExternalOutput")

    with tile.TileContext(nc) as tc, ExitStack() as ctx:
        consts = ctx.enter_context(tc.tile_pool(name="consts", bufs=1))
        xpool = ctx.enter_context(tc.tile_pool(name="xpool", bufs=1))
        wpool = ctx.enter_context(tc.tile_pool(name="wpool", bufs=2))
        proj = ctx.enter_context(tc.tile_pool(name="proj", bufs=1))
        bpool = ctx.enter_context(tc.tile_pool(name="bpool", bufs=2))
        ps = ctx.enter_context(tc.tile_pool(name="ps", bufs=4, space="PSUM"))
        pst = ctx.enter_context(tc.tile_pool(name="pst", bufs=2, space="PSUM"))

        # --- constants ---
        ident = consts.tile([P, P], BF16)
        make_identity(nc, ident[:])
        bq_sb = consts.tile([P, NET], F32)
        nc.sync.dma_start(out=bq_sb[:], in_=bq_d[:])
        bv_sb = consts.tile([P, D], F32)
        nc.gpsimd.dma_start(out=bv_sb[:], in_=bv_d[:].to_broadcast([P, D]))

        # --- load xT (d-major activations) ---
        xt_sb = xpool.tile([P, NDT * S], BF16)
        for dt in range(NDT):
            nc.sync.dma_start(
                out=xt_sb[:, dt * S:(dt + 1) * S],
                in_=xt_d[dt * P:(dt + 1) * P, :],
            )

        def load_w(dram):
            w_sb = wpool.tile([P, NDT * D], BF16, tag="w")
            for dt in range(NDT):
                nc.sync.dma_start(
                    out=w_sb[:, dt * D:(dt + 1) * D],
                    in_=dram[dt * P:(dt + 1) * P, :],
                )
            return w_sb

        # --- phase A: projections ---
        wq_sb = load_w(wq_d)
        qt_sb = proj.tile([P, NET * SQ], BF16)  # Qt[e, q], e-tile major
        for et in range(NET):
            for qc in range(NQC):
                psum = ps.tile([P, 512], F32)
                for dt in range(NDT):
                    nc.tensor.matmul(
                        psum[:],
                        lhsT=wq_sb[:, dt * D + et * P: dt * D + (et + 1) * P],
                        rhs=xt_sb[:, dt * S + qc * 512: dt * S + qc * 512 + 512],
                        start=(dt == 0),
                        stop=(dt == NDT - 1),
                    )
                nc.scalar.activation(
                    qt_sb[:, et * SQ + qc * 512: et * SQ + qc * 512 + 512],
                    psum[:], AF.Identity, bias=bq_sb[:, et:et + 1], scale=1.0,
                )

        wk_sb = load_w(wk_d)
        kt_sb = proj.tile([P, NET * S], BF16)  # Kt[e, s], e-tile major
        for et in range(NET):
            for sc in range(NKC):
                psum = ps.tile([P, 512], F32)
                for dt in range(NDT):
                    nc.tensor.matmul(
                        psum[:],
                        lhsT=wk_sb[:, dt * D + et * P: dt * D + (et + 1) * P],
                        rhs=xt_sb[:, dt * S + sc * 512: dt * S + sc * 512 + 512],
                        start=(dt == 0),
                        stop=(dt == NDT - 1),
                    )
                nc.scalar.copy(
                    kt_sb[:, et * S + sc * 512: et * S + sc * 512 + 512],
                    psum[:],
                )

        wv_sb = load_w(wv_d)
        v_sb = proj.tile([P, NST * D], BF16)  # V[s, e], s-tile major (no bias)
        for st in range(NST):
            for ec in range(NEC):
                psum = ps.tile([P, 512], F32)
                for dt in range(NDT):
                    nc.tensor.matmul(
                        psum[:],
                        lhsT=xt_sb[:, dt * S + st * P: dt * S + (st + 1) * P],
                        rhs=wv_sb[:, dt * D + ec * 512: dt * D + ec * 512 + 512],
                        start=(dt == 0),
                        stop=(dt == NDT - 1),
                    )
                nc.scalar.copy(
                    v_sb[:, st * D + ec * 512: st * D + ec * 512 + 512],
                    psum[:],
                )

        # --- phase B: attention, software-pipelined over query tiles ---
        def emit_scores(qt):
            attn_sb = bpool.tile([P, S], BF16, tag="attn")
            den4 = bpool.tile([P, NKC], F32, tag="den4")
            for kc in range(NKC):
                psum = ps.tile([P, 512], F32)
                for et in range(NET):
                    nc.tensor.matmul(
                        psum[:],
                        lhsT=qt_sb[:, et * SQ + qt * P: et * SQ + (qt + 1) * P],
                        rhs=kt_sb[:, et * S + kc * 512: et * S + kc * 512 + 512],
                        start=(et == 0),
                        stop=(et == NET - 1),
                    )
                nc.scalar.activation(
                    attn_sb[:, kc * 512:(kc + 1) * 512], psum[:],
                    AF.Exp, bias=0.0, scale=SCALE,
                    accum_out=den4[:, kc:kc + 1],
                )
            return attn_sb, den4

        def emit_out(qt, attn_sb, den4):
            den1 = bpool.tile([P, 1], F32, tag="den1")
            nc.vector.tensor_reduce(
                den1[:], den4[:], axis=mybir.AxisListType.X, op=mybir.AluOpType.add
            )
            recip = bpool.tile([P, 1], F32, tag="recip")
            nc.vector.reciprocal(recip[:], den1[:])
            attnT = bpool.tile([P, NST * P], BF16, tag="attnT")
            for ks in range(NST):
                pt = pst.tile([P, P], BF16)
                nc.tensor.transpose(pt[:], attn_sb[:, ks * P:(ks + 1) * P], ident[:])
                nc.vector.tensor_copy(attnT[:, ks * P:(ks + 1) * P], pt[:])
            out_sb = bpool.tile([P, D], F32, tag="osb")
            for ec in range(NEC):
                psum = ps.tile([P, 512], F32)
                for ks in range(NST):
                    nc.tensor.matmul(
                        psum[:],
                        lhsT=attnT[:, ks * P:(ks + 1) * P],
                        rhs=v_sb[:, ks * D + ec * 512: ks * D + ec * 512 + 512],
                        start=(ks == 0),
                        stop=(ks == NST - 1),
                    )
                sl = slice(ec * 512, (ec + 1) * 512)
                nc.scalar.activation(
                    out_sb[:, sl], psum[:], AF.Identity, bias=0.0, scale=recip[:],
                )
                nc.vector.tensor_add(out_sb[:, sl], out_sb[:, sl], bv_sb[:, sl])
            nc.sync.dma_start(out=out_d[qt * P:(qt + 1) * P, :], in_=out_sb[:])

        prev = None
        for qt in range(NQT):
            cur = (qt, *emit_scores(qt))
            if prev is not None:
                emit_out(*prev)
            prev = cur
        emit_out(*prev)

    nc.compile()
    return nc


def get_program():
    global _PROGRAM
    if _PROGRAM is None:
        _PROGRAM = _build_program()
    return _PROGRAM


def make_in_maps(x, Wq, bq, Wk, bk, Wv, bv):
    """Host-side sharding/layout prep. bk is intentionally unused (softmax
    shift invariance along the key axis)."""
    x = np.asarray(x, dtype=np.float32)
    wq_t = np.asarray(Wq, dtype=np.float32).T.astype(NPBF16)
    wk_t = np.asarray(Wk, dtype=np.float32).T.astype(NPBF16)
    wv_t = np.asarray(Wv, dtype=np.float32).T.astype(NPBF16)
    bq2 = np.ascontiguousarray(
        np.asarray(bq, dtype=np.float32).reshape(NET, P).T
    )
    bv2 = np.asarray(bv, dtype=np.float32).reshape(1, D)

    in_maps = []
    for c in range(NCORES):
        b, h = divmod(c, 2)
        xt = x[b].T.astype(NPBF16)  # [D, S]
        if h == 1:
            xt = np.concatenate([xt[:, SQ:], xt[:, :SQ]], axis=1)
        in_maps.append({
            "xt": np.ascontiguousarray(xt),
            "wq": wq_t, "wk": wk_t, "wv": wv_t,
            "bq": bq2, "bv": bv2,
        })
    return in_maps


def assemble(results):
    out = np.empty((B, S, D), dtype=np.float32)
    for c in range(NCORES):
        b, h = divmod(c, 2)
        out[b, h * SQ:(h + 1) * SQ, :] = results[c]["out"]
    return out


def kernel(x, Wq, bq, Wk, bk, Wv, bv, _trace=False, _trace_kwargs=None):
    from concourse.bass_utils import run_bass_kernel_spmd

    nc = get_program()
    in_maps = make_in_maps(x, Wq, bq, Wk, bk, Wv, bv)
    res = run_bass_kernel_spmd(
        nc, in_maps, list(range(NCORES)), trace=_trace, **(_trace_kwargs or {})
    )
    out = assemble(res.results)
    if _trace:
        kernel.last_results = res
    return out
